# revision 1
# baseline (speedup 1.0000x reference)
"""Trainium2 Bass kernel for nn_DistanceLayer (gaussian-prior distance attention).

Math: out[b,i] = sum_j softmax_j(q_i.k_j * MD^-0.5 * prior(j-i))[j] * (j-i)

Key observation: the gaussian prior (std=1) underflows so fast in f32 that
for |j-i| outside a small band the f32 score is exactly 0, so exp(score)
is exactly 1.0.  The softmax row then consists of a small band of
"interesting" values plus a uniform far field whose sums are known in
closed form.  We therefore compute only a narrow window of scores around
the diagonal on the PE and fold the far field in with exact host-side
constants:

    T0_i = (N - win) + sum_window exp(s)            (denominator)
    T1_i = C1_i + sum_window exp(s)*c + ws_i * sum_window exp(s)
    out_i = T1_i / T0_i - i

where C1_i = sum_all_j j - sum_window_i j (exact integers < 2^24, exact in
f32) and ws_i is the window start of row i's 64-row half-tile.  In-window
far entries have score exactly 0 (prior premultiplied in, 0 outside the
band) and contribute exp(0)=1, which the constants account for.

Layout: rows are processed as 64-row halves packed two-per-partition-dim
(windows stay narrow: win = 64 + band + pad), and two 128-row tiles are
batched per postprocessing pass ([P, 2*win] multiply/exp, 3D reduces for
the per-tile sums) to amortize fixed per-op engine costs.

Sharding: pure data-parallel over batch B=8 across the 8 cores; each core
holds the full (small) QK weights and computes its own [N] output row.
"""

import sys

sys.path.insert(0, "/opt/trn_rl_repo")

import ml_dtypes
import numpy as np

import concourse.bacc as bacc
import concourse.tile as tile
from concourse import mybir
from concourse.bass_utils import run_bass_kernel_spmd

B, N, D, MD = 8, 2048, 256, 128
NCORES = 8
P = 128
HR = P // 2  # 64-row half-tiles
NT = N // P  # 16 row tiles
NPAIR = NT // 2  # 8 postprocessing pairs
DCH = D // P  # 2 contraction chunks for the projections
PROJ_CHUNK = 512
NPC = N // PROJ_CHUNK  # 4 projection column chunks
PI = 3.1415926  # matches reference
F32 = mybir.dt.float32
BF16 = mybir.dt.bfloat16

_cache = {}
# exposed for test harness profiling: (nc, in_maps)
last_run = None


def _plan_band(prior_mean, prior_std):
    """f32 prior over every offset, exactly as the reference computes it,
    and the band of offsets whose scores can round exp() away from 1.0."""
    d = np.arange(-(N - 1), N, dtype=np.float32)
    ps = np.float32(prior_std)
    pm = np.float32(prior_mean)
    prior = (
        np.float32(1.0)
        / ps
        / np.sqrt(np.float32(2.0) * np.float32(PI))
        * np.exp(np.float32(-0.5) * (d - pm) ** 2 / ps**2)
    ).astype(np.float32)
    # |score| <= |prior| * |q.k*scale| ; bound the latter by 1024 (actual
    # max is ~7 for these glorot inputs).  exp(x) rounds to 1.0f for
    # |x| < 2^-26; use 2^-27 for margin.
    sig = np.abs(prior) * 1024.0 >= 2.0**-27
    if not sig.any():
        dlo, dhi = 0, 0
    else:
        dlo = int(d[sig].min())
        dhi = int(d[sig].max())
    return prior, dlo, dhi


def _window_geometry(dlo, dhi):
    """Per-64-row-half window starts ws2[32] plus deduplicated per-pair
    prior patterns.  Pattern key for pair g (tiles 2g, 2g+1) is the tuple
    of its four half-window offsets relative to the pair's base row."""
    span = dhi - dlo
    win = HR + span + 1
    win = max(80, ((win + 15) // 16) * 16)
    assert win <= 512, f"prior band too wide for banded kernel: {dlo}..{dhi}"
    extra = win - (HR + span)
    ws2 = []
    for h in range(2 * NT):
        ws = min(max(h * HR + dlo - extra // 2, 0), N - win)
        lo_need = max(0, h * HR + dlo)
        hi_need = min(N - 1, h * HR + HR - 1 + dhi)
        assert ws <= lo_need and hi_need < ws + win, (h, ws, lo_need, hi_need)
        ws2.append(ws)
    pair_keys = []
    for g in range(NPAIR):
        base = 2 * P * g
        pair_keys.append(tuple(ws2[4 * g + i] - base for i in range(4)))
    key_vals = sorted(set(pair_keys))
    key_idx = [key_vals.index(k) for k in pair_keys]
    return win, ws2, key_vals, key_idx


def _build(win, ws2, key_idx, n_pat):
    nc = bacc.Bacc()

    # f32 consts: bq | bk | c1 | wsm | ii | j0pair ; bf16: pair prior patterns
    CW = 2 + 3 * NT + 2 * win
    O_BQ, O_BK = 0, 1
    O_C1 = 2
    O_WS = O_C1 + NT
    O_II = O_WS + NT
    O_J0 = O_II + NT
    CW16 = n_pat * 2 * win

    w2_d = nc.dram_tensor("w2", [P, 2 * DCH * MD], BF16, kind="ExternalInput")
    xt_d = nc.dram_tensor("xt", [NPC, P, DCH * PROJ_CHUNK], BF16, kind="ExternalInput")
    cs_d = nc.dram_tensor("cst", [P, CW], F32, kind="ExternalInput")
    c16_d = nc.dram_tensor("cst16", [P, CW16], BF16, kind="ExternalInput")
    y_d = nc.dram_tensor("y", [P, NT], F32, kind="ExternalOutput")

    with tile.TileContext(nc) as tc:
        with (
            tc.tile_pool(name="const", bufs=1) as const,
            tc.tile_pool(name="psum_proj", bufs=3, space="PSUM") as psum_proj,
            tc.tile_pool(name="psum_band", bufs=3, space="PSUM") as psum_band,
            tc.tile_pool(name="band_sp", bufs=2) as sp_pool,
            tc.tile_pool(name="band_e", bufs=2) as e_pool,
            tc.tile_pool(name="band_ej", bufs=2) as ej_pool,
            tc.tile_pool(name="comb", bufs=1) as comb,
        ):
            # ---- engine warmups (run while DMAs are in flight) ----
            # PE: junk matmuls keep the PE busy until the input DMAs land,
            # flipping the HAM clock gate to 8/8 before the real matmuls.
            # ACT: one tiny Exp pulls the 1.3us ACT_TABLE_LOAD off the
            # critical path.
            wtile = const.tile([P, PROJ_CHUNK], BF16, tag="warm_w")
            nc.vector.memset(wtile, 0.0)
            for _ in range(7):
                wps = psum_proj.tile([P, PROJ_CHUNK], F32, tag="proj")
                nc.tensor.matmul(
                    wps,
                    lhsT=wtile[:, :P],
                    rhs=wtile[:, :PROJ_CHUNK],
                    start=True,
                    stop=True,
                )
            wact_in = const.tile([P, 1], F32, tag="warm_a")
            nc.vector.memset(wact_in, 0.0)
            wact_out = const.tile([P, 1], F32, tag="warm_ao")
            nc.scalar.activation(
                out=wact_out, in_=wact_in, func=mybir.ActivationFunctionType.Exp
            )

            # ---- input DMAs; first ones go on the scalar queue so their
            # descriptor generation runs parallel to sync's preamble ----
            w2_s = const.tile([P, 2 * DCH * MD], BF16, tag="w2")
            nc.scalar.dma_start(out=w2_s, in_=w2_d[:, :])
            xts = []
            for i in range(NPC):
                t = const.tile([P, DCH * PROJ_CHUNK], BF16, tag=f"xt{i}")
                xts.append(t)
            nc.scalar.dma_start(out=xts[0], in_=xt_d[0])
            cs_s = const.tile([P, CW], F32, tag="cst")
            nc.scalar.dma_start(out=cs_s, in_=cs_d[:, :])
            c16_s = const.tile([P, CW16], BF16, tag="cst16")
            nc.scalar.dma_start(out=c16_s, in_=c16_d[:, :])
            for i in range(1, NPC):
                nc.sync.dma_start(out=xts[i], in_=xt_d[i])

            qT = const.tile([P, N], BF16, tag="qT")
            kT = const.tile([P, N], BF16, tag="kT")
            sum_e = const.tile([P, NT], F32, tag="sum_e")
            sum_ec = const.tile([P, NT], F32, tag="sum_ec")

            # ---- band pair: tiles 2g, 2g+1 share one [P, 2*win] pass ----
            def emit_pair(g):
                ps_s = psum_band.tile([P, 2 * win], F32, tag="band")
                for tb in range(2):  # tile within pair
                    t = 2 * g + tb
                    for hb in range(2):  # 64-row half on partitions
                        ws = ws2[2 * t + hb]
                        nc.tensor.matmul(
                            ps_s[hb * HR : (hb + 1) * HR, tb * win : (tb + 1) * win],
                            lhsT=qT[:, t * P + hb * HR : t * P + (hb + 1) * HR],
                            rhs=kT[:, ws : ws + win],
                            start=True,
                            stop=True,
                        )
                oi = key_idx[g]
                sp_t = sp_pool.tile([P, 2 * win], F32, tag="sp")
                nc.vector.tensor_mul(
                    sp_t, ps_s, c16_s[:, oi * 2 * win : (oi + 1) * 2 * win]
                )
                e_t = e_pool.tile([P, 2 * win], F32, tag="e")
                nc.scalar.activation(
                    out=e_t, in_=sp_t, func=mybir.ActivationFunctionType.Exp
                )
                ej_t = ej_pool.tile([P, 2 * win], F32, tag="ej")
                mul_eng = nc.vector if g == NPAIR - 1 else nc.gpsimd
                mul_eng.tensor_mul(ej_t, e_t, cs_s[:, O_J0 : O_J0 + 2 * win])
                nc.vector.tensor_reduce(
                    out=sum_e[:, 2 * g : 2 * g + 2],
                    in_=e_t[:].rearrange("p (t w) -> p t w", w=win),
                    axis=mybir.AxisListType.X,
                    op=mybir.AluOpType.add,
                )
                nc.vector.tensor_reduce(
                    out=sum_ec[:, 2 * g : 2 * g + 2],
                    in_=ej_t[:].rearrange("p (t w) -> p t w", w=win),
                    axis=mybir.AxisListType.X,
                    op=mybir.AluOpType.add,
                )

            # pair g needs both projections evicted through this chunk:
            def pair_chunk(g):
                hi = max(min(ws2[h] + win, N) for h in range(4 * g, 4 * g + 4))
                return max((2 * g + 1) // (PROJ_CHUNK // P), (hi - 1) // PROJ_CHUNK)

            pairs_after = {n4: [] for n4 in range(NPC)}
            for g in range(NPAIR):
                pairs_after[pair_chunk(g)].append(g)

            # ---- projections; chunk0 evictions split across ACT+DVE for
            # the fastest band unlock, later chunks all on ACT (the band
            # postprocessing now loads DVE+GpSimd more than ACT) ----
            def emit_chunk(n4, split_evict=False):
                for pj in range(2):  # 0=q, 1=k
                    b_s = cs_s[:, O_BQ + pj : O_BQ + pj + 1]
                    dstT = (qT, kT)[pj]
                    ps_t = psum_proj.tile([P, PROJ_CHUNK], F32, tag="proj")
                    for c in range(DCH):
                        nc.tensor.matmul(
                            ps_t,
                            lhsT=w2_s[:, (2 * pj + c) * MD : (2 * pj + c + 1) * MD],
                            rhs=xts[n4][:, c * PROJ_CHUNK : (c + 1) * PROJ_CHUNK],
                            start=(c == 0),
                            stop=(c == DCH - 1),
                        )
                    lo = n4 * PROJ_CHUNK
                    if split_evict:
                        half = PROJ_CHUNK // 2
                        nc.vector.tensor_scalar_add(
                            dstT[:, lo : lo + half], ps_t[:, :half], b_s
                        )
                        nc.scalar.activation(
                            out=dstT[:, lo + half : lo + PROJ_CHUNK],
                            in_=ps_t[:, half:],
                            func=mybir.ActivationFunctionType.Identity,
                            bias=b_s,
                            scale=1.0,
                        )
                    else:
                        nc.scalar.activation(
                            out=dstT[:, lo : lo + PROJ_CHUNK],
                            in_=ps_t,
                            func=mybir.ActivationFunctionType.Identity,
                            bias=b_s,
                            scale=1.0,
                        )

            # shift-by-one: pair MMs are emitted after the NEXT chunk's
            # matmuls so their evictions are already done (engine queues
            # are FIFO; a waiting matmul would stall the whole PE queue).
            emit_chunk(0, split_evict=True)
            emit_chunk(1)
            band_plan = []
            for n4 in range(2, NPC + 2):
                for g in pairs_after[n4 - 2]:
                    band_plan.append(("pair", g))
                if n4 < NPC:
                    band_plan.append(("chunk", n4))


            # ---- combine: out = (c1 + sum_ec + ws*sum_e)/(N-win+sum_e) - i ----
            c1_s = cs_s[:, O_C1 : O_C1 + NT]
            ws_s = cs_s[:, O_WS : O_WS + NT]
            ii_s = cs_s[:, O_II : O_II + NT]
            outv2 = comb.tile([P, NT], F32, tag="outv2")

            def emit_combine(sl):
                w = sl.stop - sl.start
                t0 = comb.tile([P, w], F32, tag="t0")
                nc.vector.tensor_scalar_add(t0, sum_e[:, sl], float(N - win))
                rec = comb.tile([P, w], F32, tag="rec")
                nc.vector.reciprocal(rec, t0)
                tmp = comb.tile([P, w], F32, tag="tmp")
                nc.vector.tensor_mul(tmp, ws_s[:, sl], sum_e[:, sl])
                num = comb.tile([P, w], F32, tag="num")
                nc.vector.tensor_add(num, c1_s[:, sl], sum_ec[:, sl])
                num2 = comb.tile([P, w], F32, tag="num2")
                nc.vector.tensor_add(num2, num, tmp)
                outv = comb.tile([P, w], F32, tag="outv")
                nc.vector.tensor_mul(outv, num2, rec)
                nc.vector.tensor_sub(outv2[:, sl], outv, ii_s[:, sl])

            # first-half combine hides under the last pairs
            for kind, v in band_plan:
                if kind == "pair":
                    emit_pair(v)
                    if v == NPAIR - 2:
                        emit_combine(slice(0, 8))
                else:
                    emit_chunk(v)
            emit_combine(slice(8, NT))
            nc.sync.dma_start(out=y_d[:, :], in_=outv2)

    nc.finalize()
    return nc


def kernel(x, Wq, bq, Wk, bk, prior_mean, prior_std):
    global last_run
    x = np.asarray(x, dtype=np.float32)
    Wq = np.asarray(Wq, dtype=np.float32)
    Wk = np.asarray(Wk, dtype=np.float32)
    bq = np.asarray(bq, dtype=np.float32)
    bk = np.asarray(bk, dtype=np.float32)

    prior, dlo, dhi = _plan_band(
        float(np.asarray(prior_mean)[0]), float(np.asarray(prior_std)[0])
    )
    win, ws2, key_vals, key_idx = _window_geometry(dlo, dhi)
    n_pat = len(key_vals)

    key = (win, tuple(ws2), tuple(key_idx))
    if key not in _cache:
        _cache[key] = _build(win, ws2, key_idx, n_pat)
    nc = _cache[key]

    bf = ml_dtypes.bfloat16
    scale = np.float32(MD**-0.5)

    # prior*scale pair patterns: [P, 2*win] per distinct 4-offset key.
    # value[p, tb*win + c] = prior[c + rel_ws[tb, hb] - 128*tb - p] * scale
    # where hb selects by partition half (p >= 64).
    p_idx = np.arange(P)[:, None]
    c_idx = np.arange(win)[None, :]
    pmat = np.zeros((P, n_pat * 2 * win), np.float32)
    for ki, rel in enumerate(key_vals):
        for tb in range(2):
            relcol = np.where(np.arange(P) < HR, rel[2 * tb], rel[2 * tb + 1])[:, None]
            dm = c_idx + relcol - 128 * tb - p_idx
            pmat[:, ki * 2 * win + tb * win : ki * 2 * win + (tb + 1) * win] = np.where(
                (dm >= dlo) & (dm <= dhi), prior[dm + N - 1] * scale, np.float32(0.0)
            ).astype(np.float32)

    sumj_all = float(N * (N - 1) // 2)
    c1 = np.zeros((P, NT), np.float32)
    wsm = np.zeros((P, NT), np.float32)
    ii = np.zeros((P, NT), np.float32)
    half_sel = np.arange(P) >= HR
    for t in range(NT):
        wsa, wsb = ws2[2 * t], ws2[2 * t + 1]
        wsv = np.where(half_sel, float(wsb), float(wsa))
        c1[:, t] = sumj_all - (win * wsv + win * (win - 1) // 2)
        wsm[:, t] = wsv
        ii[:, t] = t * P + np.arange(P)

    # consts: f32 = bq | bk | c1 | wsm | ii | j0pair ; bf16 = pair patterns
    j0pair = np.broadcast_to(
        np.tile(np.arange(win, dtype=np.float32), 2), (P, 2 * win)
    )
    cst = np.ascontiguousarray(
        np.concatenate(
            [bq.reshape(P, 1), bk.reshape(P, 1), c1, wsm, ii, j0pair], axis=1
        ).astype(np.float32)
    )
    cst16 = np.ascontiguousarray(pmat.astype(bf))

    # weights: wq chunks then wk chunks, [P, 4*MD]
    wq_h = Wq.reshape(DCH, P, MD).transpose(1, 0, 2).reshape(P, DCH * MD)
    wk_h = Wk.reshape(DCH, P, MD).transpose(1, 0, 2).reshape(P, DCH * MD)
    w2_h = np.ascontiguousarray(np.concatenate([wq_h, wk_h], axis=1)).astype(bf)

    in_maps = []
    for core in range(NCORES):
        xb = x[core]  # [N, D]
        # xt[n4, p, c*512 + j] = x[n4*512 + j, c*128 + p]
        xt_h = np.ascontiguousarray(
            xb.T.reshape(DCH, P, NPC, PROJ_CHUNK)
            .transpose(2, 1, 0, 3)
            .reshape(NPC, P, DCH * PROJ_CHUNK)
        ).astype(bf)
        in_maps.append({"xt": xt_h, "w2": w2_h, "cst": cst, "cst16": cst16})

    res = run_bass_kernel_spmd(nc, in_maps, list(range(NCORES)))
    last_run = (nc, in_maps)
    # y[p, t] = out[128t + p]  ->  out = y.T.flatten()
    out = np.stack(
        [res.results[c]["y"].T.reshape(-1) for c in range(NCORES)], axis=0
    )
    return out.astype(np.float32)



# revision 5
# speedup vs baseline: 1.0568x; 1.0568x over previous
"""Trainium2 Bass kernel for nn_DistanceLayer (gaussian-prior distance attention).

Math: out[b,i] = sum_j softmax_j(q_i.k_j * MD^-0.5 * prior(j-i))[j] * (j-i)

The gaussian prior (std=1) underflows so fast in f32 that outside a small
band of offsets the f32 score is exactly 0, so exp(score) is exactly 1.0.
Each softmax row is a narrow band of interesting values plus a uniform far
field with closed-form sums.  We compute a [128, win] window of scores per
128-row tile on the PE and fold the far field in with exact constants:

    T0_i = (N - win) + sum_win e           (denominator)
    out_i = (A_i + sum_win e*j0 + B_i * sum_win e) / T0_i

with A_i = sum_all j - sum_win_i j - i*(N-win)  (exact ints in f32),
B_i = ws_i - i, j0 the window-local column index.

v2 layout/scheduling:
- fp8 (e4m3) x and weights; each projection chunk is a single DoubleRow
  matmul contracting all 256 input dims at 2 elem/cell; q and k land in one
  [P, 1024] PSUM tile evicted to SBUF bf16 in one op (zero-bias fast path).
- full 128-row band windows (win = 128 + band + pad), one matmul per tile.
- postproc per pair of tiles: DVE multiplies scores by the premultiplied
  prior pattern (into PSUM), ACT exp's each tile window (PSUM src is
  cheaper there) with accum_out giving sum_e for free and bf16 e output,
  DVE's fused tensor_tensor_reduce gives sum_e*j0 in one bf16 2x op.
- combine runs on GpSimd (SBUF-only engine) except the reciprocal.
- inputs split over the sync + scalar DMA queues ordered by need time.

Sharding: pure data-parallel over batch B=8 across the 8 cores.
"""

import sys

sys.path.insert(0, "/opt/trn_rl_repo")

import ml_dtypes
import numpy as np

import concourse.bacc as bacc
import concourse.tile as tile
from concourse import mybir
from concourse.bass_utils import run_bass_kernel_spmd

B, N, D, MD = 8, 2048, 256, 128
NCORES = 8
P = 128
NT = N // P  # 16 row tiles
NPAIR = NT // 2  # 8 postprocessing pairs
DCH = D // P  # 2 contraction chunks (fused by DoubleRow)
PROJ_CHUNK = 512
NPC = N // PROJ_CHUNK  # 4 projection column chunks
PI = 3.1415926  # matches reference
F32 = mybir.dt.float32
BF16 = mybir.dt.bfloat16
F8 = mybir.dt.float8e4
AL = mybir.AluOpType
AF = mybir.ActivationFunctionType
N_WARM = 8  # PE clock-ramp junk matmuls

_cache = {}
# exposed for test harness profiling: (nc, in_maps)
last_run = None


def _plan_band(prior_mean, prior_std):
    """f32 prior over every offset, exactly as the reference computes it,
    and the band of offsets whose scores can round exp() away from 1.0."""
    d = np.arange(-(N - 1), N, dtype=np.float32)
    ps = np.float32(prior_std)
    pm = np.float32(prior_mean)
    prior = (
        np.float32(1.0)
        / ps
        / np.sqrt(np.float32(2.0) * np.float32(PI))
        * np.exp(np.float32(-0.5) * (d - pm) ** 2 / ps**2)
    ).astype(np.float32)
    # |score| <= |prior| * |q.k*scale| ; bound the latter by 1024 (actual
    # max is ~7 for these glorot inputs).  exp(x) rounds to 1.0f for
    # |x| < 2^-26; use 2^-27 for margin.
    sig = np.abs(prior) * 1024.0 >= 2.0**-27
    if not sig.any():
        dlo, dhi = 0, 0
    else:
        dlo = int(d[sig].min())
        dhi = int(d[sig].max())
    return prior, dlo, dhi


def _window_geometry(dlo, dhi):
    """Per-tile window starts ws[NT] plus a concatenated pattern sequence
    where each pair's two tile patterns sit at consecutive positions."""
    span = dhi - dlo
    win = P + span + 1
    win = ((win + 15) // 16) * 16
    assert win <= 256, f"prior band too wide for banded kernel: {dlo}..{dhi}"
    extra = win - (P + span)
    ws = [min(max(t * P + dlo - extra // 2, 0), N - win) for t in range(NT)]
    for t in range(NT):
        lo_need = max(0, t * P + dlo)
        hi_need = min(N - 1, t * P + P - 1 + dhi)
        assert ws[t] <= lo_need and hi_need < ws[t] + win, (t, ws[t])
    keys = [ws[t] - P * t for t in range(NT)]
    seq = []
    pair_off = []
    for g in range(NPAIR):
        combo = (keys[2 * g], keys[2 * g + 1])
        found = None
        for i in range(len(seq) - 1):
            if (seq[i], seq[i + 1]) == combo:
                found = i
                break
        if found is None:
            if seq and seq[-1] == combo[0]:
                seq.append(combo[1])
            else:
                seq.extend(combo)
            found = len(seq) - 2
        pair_off.append(found)
    return win, ws, seq, pair_off


def _build(win, ws, n_seq, pair_off, has_bias):
    nc = bacc.Bacc()

    # f32 consts: bq | bk | A | B ; bf16: prior patterns | j0
    CW = 2 + 2 * NT
    O_BQ, O_BK = 0, 1
    O_A = 2
    O_B = O_A + NT
    O_J0 = n_seq * win  # in cst16

    w2_d = nc.dram_tensor("w2", [P, 2 * DCH * MD], F8, kind="ExternalInput")
    xt_d = nc.dram_tensor("xt", [NPC, P, DCH * PROJ_CHUNK], F8, kind="ExternalInput")
    cs_d = nc.dram_tensor("cst", [P, CW], F32, kind="ExternalInput")
    c16_d = nc.dram_tensor("cst16", [P, (n_seq + 1) * win], BF16, kind="ExternalInput")
    y_d = nc.dram_tensor("y", [P, NT], F32, kind="ExternalOutput")

    with tile.TileContext(nc) as tc:
        with (
            tc.tile_pool(name="const", bufs=1) as const,
            tc.tile_pool(name="psum_proj", bufs=2, space="PSUM") as psum_proj,
            tc.tile_pool(name="psum_band", bufs=2, space="PSUM") as psum_band,
            tc.tile_pool(name="psum_sp", bufs=2, space="PSUM") as psum_sp,
            tc.tile_pool(name="band_e", bufs=2) as e_pool,
            tc.tile_pool(name="band_ej", bufs=2) as ej_pool,
            tc.tile_pool(name="comb", bufs=1) as comb,
        ):
            # ---- engine warmups (run while DMAs are in flight) ----
            # PE: junk matmuls ramp the HAM clock gate to 8/8 before the
            # real matmuls.  ACT: one tiny Exp pulls the 1.3us
            # ACT_TABLE_LOAD off the critical path.
            wtile = const.tile([P, 256], BF16, tag="warm_w")
            nc.vector.memset(wtile, 0.0)
            wact_in = const.tile([P, 1], F32, tag="warm_a")
            nc.vector.memset(wact_in, 0.0)
            for _ in range(N_WARM):
                wps = psum_band.tile([P, 2 * win], F32, tag="band")
                nc.tensor.matmul(
                    wps[:, :256], lhsT=wtile[:, :P], rhs=wtile, start=True, stop=True
                )
            wact_out = const.tile([P, 1], F32, tag="warm_ao")
            nc.scalar.activation(out=wact_out, in_=wact_in, func=AF.Exp)

            # ---- input DMAs, ordered by need time ----
            # sync queue: w2, xt0, cst (+ the y output later)
            # scalar queue: cst16, xt1, xt2, xt3
            w2_s = const.tile([P, 2 * DCH * MD], F8, tag="w2")
            nc.sync.dma_start(out=w2_s, in_=w2_d[:, :])
            xts = []
            for i in range(NPC):
                t = const.tile([P, DCH * PROJ_CHUNK], F8, tag=f"xt{i}")
                xts.append(t)
            nc.sync.dma_start(out=xts[0], in_=xt_d[0])
            cs_s = const.tile([P, CW], F32, tag="cst")
            nc.sync.dma_start(out=cs_s, in_=cs_d[:, :])
            c16_s = const.tile([P, (n_seq + 1) * win], BF16, tag="cst16")
            nc.scalar.dma_start(out=c16_s, in_=c16_d[:, :])
            for i in range(1, NPC):
                nc.scalar.dma_start(out=xts[i], in_=xt_d[i])

            # q is cols [0, N), k is cols [N, 2N)
            qkT = const.tile([P, 2 * N], BF16, tag="qkT")
            sum_e = const.tile([P, NT], F32, tag="sum_e")
            sum_ec = const.tile([P, NT], F32, tag="sum_ec")
            outv = const.tile([P, NT], F32, tag="outv")

            # ---- projections: one DoubleRow matmul per (chunk, q/k),
            # both halves in one PSUM tile, evicted in one fused op ----
            def emit_proj(n4, evict_eng):
                ps = psum_proj.tile([P, 2 * PROJ_CHUNK], F32, tag="proj")
                rhs = xts[n4][:, :].rearrange("p (c f) -> p c f", c=DCH)
                for pj in range(2):  # 0=q, 1=k
                    lhsT = w2_s[:, pj * DCH * MD : (pj + 1) * DCH * MD].rearrange(
                        "p (c m) -> p c m", c=DCH
                    )
                    nc.tensor.matmul(
                        ps[:, pj * PROJ_CHUNK : (pj + 1) * PROJ_CHUNK],
                        lhsT=lhsT,
                        rhs=rhs,
                        start=True,
                        stop=True,
                        perf_mode=mybir.MatmulPerfMode.DoubleRow,
                    )
                lo = n4 * PROJ_CHUNK
                if not has_bias:
                    dst = qkT[:, :].rearrange("p (qk n) -> p qk n", qk=2)[
                        :, :, lo : lo + PROJ_CHUNK
                    ]
                    src = ps[:, :].rearrange("p (qk f) -> p qk f", qk=2)
                    if evict_eng == "act":
                        nc.scalar.activation(out=dst, in_=src, func=AF.Identity)
                    else:
                        nc.vector.tensor_copy(dst, src)
                else:
                    for pj in range(2):
                        b_s = cs_s[:, O_BQ + pj : O_BQ + pj + 1]
                        nc.scalar.activation(
                            out=qkT[:, pj * N + lo : pj * N + lo + PROJ_CHUNK],
                            in_=ps[:, pj * PROJ_CHUNK : (pj + 1) * PROJ_CHUNK],
                            func=AF.Identity,
                            bias=b_s,
                            scale=1.0,
                        )

            # ---- band pair: matmuls (PE) and postprocessing ----
            pair_ps = {}

            def emit_pair_mm(g):
                ps = psum_band.tile([P, 2 * win], F32, tag="band")
                for tb in range(2):
                    t = 2 * g + tb
                    nc.tensor.matmul(
                        ps[:, tb * win : (tb + 1) * win],
                        lhsT=qkT[:, t * P : (t + 1) * P],
                        rhs=qkT[:, N + ws[t] : N + ws[t] + win],
                        start=True,
                        stop=True,
                    )
                pair_ps[g] = ps

            def emit_pair_post(g):
                ps = pair_ps.pop(g)
                sp = psum_sp.tile([P, 2 * win], F32, tag="sp")
                nc.vector.tensor_mul(
                    sp, ps, c16_s[:, pair_off[g] * win : (pair_off[g] + 2) * win]
                )
                e_t = e_pool.tile([P, 2 * win], BF16, tag="e")
                for tb in range(2):
                    t = 2 * g + tb
                    nc.scalar.activation(
                        out=e_t[:, tb * win : (tb + 1) * win],
                        in_=sp[:, tb * win : (tb + 1) * win],
                        func=AF.Exp,
                        accum_out=sum_e[:, t : t + 1],
                    )
                for tb in range(2):
                    t = 2 * g + tb
                    ej = ej_pool.tile([P, win], F32, tag="ej")
                    nc.vector.affine_mul_reduce(
                        out=ej,
                        accum_out=sum_ec[:, t : t + 1],
                        in0=e_t[:, tb * win : (tb + 1) * win],
                        in1=c16_s[:, O_J0 : O_J0 + win],
                        scale=1.0,
                        bias=0.0,
                    )

            # ---- combine: out = (A + sum_ec + B*sum_e)/(N-win+sum_e) ----
            # GpSimd (SBUF-only engine) does everything but the reciprocal.
            def comb_pre(sl):
                w = sl.stop - sl.start
                t0 = comb.tile([P, w], F32, tag=f"t0{sl.start}")
                nc.gpsimd.tensor_scalar_add(t0, sum_e[:, sl], float(N - win))
                rec = comb.tile([P, w], F32, tag=f"rec{sl.start}")
                nc.vector.reciprocal(rec, t0)
                tmp = comb.tile([P, w], F32, tag=f"tmp{sl.start}")
                nc.gpsimd.tensor_mul(tmp, cs_s[:, O_B + sl.start : O_B + sl.stop],
                                     sum_e[:, sl])
                return rec, tmp

            def comb_post(sl, rec, tmp):
                w = sl.stop - sl.start
                num = comb.tile([P, w], F32, tag=f"num{sl.start}")
                nc.gpsimd.tensor_add(num, cs_s[:, O_A + sl.start : O_A + sl.stop],
                                     sum_ec[:, sl])
                num2 = comb.tile([P, w], F32, tag=f"num2{sl.start}")
                nc.gpsimd.tensor_add(num2, num, tmp)
                nc.gpsimd.tensor_mul(outv[:, sl], num2, rec)
                nc.sync.dma_start(out=y_d[:, sl], in_=outv[:, sl])

            # ---- schedule ----
            # pair g's k window lives entirely below chunk pair_chunk(g)+1
            def pair_chunk(g):
                hi = max(ws[t] + win for t in (2 * g, 2 * g + 1))
                return max((2 * g + 1) // (PROJ_CHUNK // P), (hi - 1) // PROJ_CHUNK)

            pairs_of = {c: [] for c in range(NPC)}
            for g in range(NPAIR):
                pairs_of[pair_chunk(g)].append(g)

            slA, slB = slice(0, 8), slice(8, NT)

            emit_proj(0, "dve")
            emit_proj(1, "act")
            for g in pairs_of[0]:
                emit_pair_mm(g)
            emit_proj(2, "dve")
            for g in pairs_of[0]:
                emit_pair_post(g)
            for g in pairs_of[1]:
                emit_pair_mm(g)
            emit_proj(3, "act")
            for g in pairs_of[1]:
                emit_pair_post(g)
            for g in pairs_of[2]:
                emit_pair_mm(g)
            for g in pairs_of[2]:
                emit_pair_post(g)
            for g in pairs_of[3]:
                emit_pair_mm(g)
            done = 0
            recA = tmpA = None
            for g in pairs_of[3]:
                emit_pair_post(g)
                done += 1
                if done == 1:
                    recA, tmpA = comb_pre(slA)
                elif done == 2:
                    comb_post(slA, recA, tmpA)
            recB, tmpB = comb_pre(slB)
            comb_post(slB, recB, tmpB)

    nc.finalize()
    return nc


def kernel(x, Wq, bq, Wk, bk, prior_mean, prior_std):
    global last_run
    x = np.asarray(x, dtype=np.float32)
    Wq = np.asarray(Wq, dtype=np.float32)
    Wk = np.asarray(Wk, dtype=np.float32)
    bq = np.asarray(bq, dtype=np.float32)
    bk = np.asarray(bk, dtype=np.float32)
    has_bias = bool(np.any(bq) or np.any(bk))

    prior, dlo, dhi = _plan_band(
        float(np.asarray(prior_mean)[0]), float(np.asarray(prior_std)[0])
    )
    win, ws, seq, pair_off = _window_geometry(dlo, dhi)
    n_seq = len(seq)

    key = (win, tuple(ws), tuple(pair_off), n_seq, has_bias)
    if key not in _cache:
        _cache[key] = _build(win, ws, n_seq, pair_off, has_bias)
    nc = _cache[key]

    bf = ml_dtypes.bfloat16
    f8 = ml_dtypes.float8_e4m3fn
    scale = np.float32(MD**-0.5)

    # prior*scale patterns: [P, win] per sequence position with key k:
    # pat[p, j] = prior[j + k - p] * scale (0 outside the significant band)
    p_idx = np.arange(P)[:, None]
    c_idx = np.arange(win)[None, :]
    pmat = np.zeros((P, n_seq * win), np.float32)
    for s, k in enumerate(seq):
        dm = c_idx + k - p_idx
        pmat[:, s * win : (s + 1) * win] = np.where(
            (dm >= dlo) & (dm <= dhi), prior[dm + N - 1] * scale, np.float32(0.0)
        ).astype(np.float32)

    sumj_all = float(N * (N - 1) // 2)
    ii = (np.arange(P)[:, None] + P * np.arange(NT)[None, :]).astype(np.float32)
    wsv = np.broadcast_to(np.array(ws, np.float32)[None, :], (P, NT))
    c1 = sumj_all - (win * wsv + win * (win - 1) // 2)
    A = c1 - ii * float(N - win)
    Bv = wsv - ii

    cst = np.ascontiguousarray(
        np.concatenate([bq.reshape(P, 1), bk.reshape(P, 1), A, Bv], axis=1).astype(
            np.float32
        )
    )
    j0 = np.broadcast_to(np.arange(win, dtype=np.float32)[None, :], (P, win))
    cst16 = np.ascontiguousarray(
        np.concatenate([pmat, j0], axis=1).astype(bf)
    )

    # weights: wq chunks then wk chunks, [P, 4*MD] fp8
    wq_h = Wq.reshape(DCH, P, MD).transpose(1, 0, 2).reshape(P, DCH * MD)
    wk_h = Wk.reshape(DCH, P, MD).transpose(1, 0, 2).reshape(P, DCH * MD)
    w2_h = np.clip(np.concatenate([wq_h, wk_h], axis=1), -240, 240)
    w2_h = np.ascontiguousarray(w2_h).astype(f8)

    in_maps = []
    for core in range(NCORES):
        xb = x[core]  # [N, D]
        # xt[n4, p, c*512 + j] = x[n4*512 + j, c*128 + p]
        xt_h = np.ascontiguousarray(
            np.clip(xb.T, -240, 240)
            .reshape(DCH, P, NPC, PROJ_CHUNK)
            .transpose(2, 1, 0, 3)
            .reshape(NPC, P, DCH * PROJ_CHUNK)
        ).astype(f8)
        in_maps.append({"xt": xt_h, "w2": w2_h, "cst": cst, "cst16": cst16})

    res = run_bass_kernel_spmd(nc, in_maps, list(range(NCORES)))
    last_run = (nc, in_maps)
    # y[p, t] = out[128t + p]  ->  out = y.T.flatten()
    out = np.stack(
        [res.results[c]["y"].T.reshape(-1) for c in range(NCORES)], axis=0
    )
    return out.astype(np.float32)


# revision 7
# speedup vs baseline: 1.1024x; 1.0431x over previous
"""Trainium2 Bass kernel for nn_DistanceLayer (gaussian-prior distance attention).

Math: out[b,i] = sum_j softmax_j(q_i.k_j * MD^-0.5 * prior(j-i))[j] * (j-i)

The gaussian prior (std=1) underflows so fast in f32 that outside a small
band of offsets the f32 score is exactly 0, so exp(score) is exactly 1.0.
Each softmax row is a narrow band of interesting values plus a uniform far
field with closed-form sums.  We compute a narrow window of scores around
the diagonal on the PE and fold the far field in with exact constants:

    T0_i = (N - win) + sum_win e           (denominator)
    out_i = (A_i + sum_win e*j0 + B_i * sum_win e) / T0_i

with A_i = sum_all j - sum_win_i j - i*(N-win)  (exact ints in f32),
B_i = ws_i - i, j0 the window-local column index.

Structure:
- rows processed as 64-row halves packed two-per-partition-dim (windows
  stay narrow: win = 64 + band + pad); the h0/h64 half matmuls run
  concurrently on the PE via column groups.
- fp8 (e4m3) x and weights; each projection chunk is one DoubleRow matmul
  per q/k contracting all 256 input dims at 2 elem/cell; q and k land in
  one [P, 1024] PSUM tile, evicted as two [P, 512] halves on DVE + ACT.
- postproc per pair of tiles [P, 2*win]: DVE multiplies scores by the
  premultiplied prior pattern into PSUM, ACT exp's the pair into a packed
  bf16 e|ej tile, GpSimd multiplies e by j0 into the ej half, and one
  tensor_reduce over [P, (4, win)] yields all four sums per pair.
- combine runs on GpSimd except the reciprocal; y is written in 2 DMAs.
- inputs split over the sync + scalar DMA queues ordered by need time.

Sharding: pure data-parallel over batch B=8 across the 8 cores.
"""

import sys

sys.path.insert(0, "/opt/trn_rl_repo")

import ml_dtypes
import numpy as np

import concourse.bacc as bacc
import concourse.tile as tile
from concourse import mybir
from concourse.bass_utils import run_bass_kernel_spmd

B, N, D, MD = 8, 2048, 256, 128
NCORES = 8
P = 128
HR = P // 2  # 64-row half-tiles
NT = N // P  # 16 row tiles
NPAIR = NT // 2  # 8 postprocessing pairs
DCH = D // P  # 2 contraction chunks (fused by DoubleRow)
PROJ_CHUNK = 512
NPC = N // PROJ_CHUNK  # 4 projection column chunks
PI = 3.1415926  # matches reference
F32 = mybir.dt.float32
BF16 = mybir.dt.bfloat16
F8 = mybir.dt.float8e4
AL = mybir.AluOpType
AF = mybir.ActivationFunctionType
N_WARM = 10  # PE clock-ramp junk matmuls

_cache = {}
# exposed for test harness profiling: (nc, in_maps)
last_run = None


def _plan_band(prior_mean, prior_std):
    """f32 prior over every offset, exactly as the reference computes it,
    and the band of offsets whose scores can round exp() away from 1.0."""
    d = np.arange(-(N - 1), N, dtype=np.float32)
    ps = np.float32(prior_std)
    pm = np.float32(prior_mean)
    prior = (
        np.float32(1.0)
        / ps
        / np.sqrt(np.float32(2.0) * np.float32(PI))
        * np.exp(np.float32(-0.5) * (d - pm) ** 2 / ps**2)
    ).astype(np.float32)
    sig = np.abs(prior) * 1024.0 >= 2.0**-27
    if not sig.any():
        dlo, dhi = 0, 0
    else:
        dlo = int(d[sig].min())
        dhi = int(d[sig].max())
    return prior, dlo, dhi


def _window_geometry(dlo, dhi):
    """Per-64-row-half window starts ws2[32] plus deduplicated per-pair
    prior patterns.  Pattern key for pair g (tiles 2g, 2g+1) is the tuple
    of its four half-window offsets relative to the pair's base row."""
    span = dhi - dlo
    win = HR + span + 1
    win = max(80, ((win + 15) // 16) * 16)
    assert win <= 256, f"prior band too wide for banded kernel: {dlo}..{dhi}"
    extra = win - (HR + span)
    ws2 = []
    for h in range(2 * NT):
        ws = min(max(h * HR + dlo - extra // 2, 0), N - win)
        lo_need = max(0, h * HR + dlo)
        hi_need = min(N - 1, h * HR + HR - 1 + dhi)
        assert ws <= lo_need and hi_need < ws + win, (h, ws, lo_need, hi_need)
        ws2.append(ws)
    pair_keys = []
    for g in range(NPAIR):
        base = 2 * P * g
        pair_keys.append(tuple(ws2[4 * g + i] - base for i in range(4)))
    key_vals = sorted(set(pair_keys))
    key_idx = [key_vals.index(k) for k in pair_keys]
    return win, ws2, key_vals, key_idx


def _build(win, ws2, key_idx, n_pat, has_bias):
    nc = bacc.Bacc()

    # f32 consts: A | B (+ biases) ; bf16: pair patterns | j0pair
    CW = 2 + 2 * NT
    O_BQ, O_BK = 0, 1
    O_A = 2
    O_B = O_A + NT
    O_J0 = n_pat * 2 * win  # j0pair offset inside cst16

    w2_d = nc.dram_tensor("w2", [P, 2 * DCH * MD], F8, kind="ExternalInput")
    xt_d = nc.dram_tensor("xt", [NPC, P, DCH * PROJ_CHUNK], F8, kind="ExternalInput")
    cs_d = nc.dram_tensor("cst", [P, CW], F32, kind="ExternalInput")
    c16_d = nc.dram_tensor(
        "cst16", [P, (n_pat + 1) * 2 * win], BF16, kind="ExternalInput"
    )
    y_d = nc.dram_tensor("y", [P, NT], F32, kind="ExternalOutput")

    with tile.TileContext(nc) as tc:
        with (
            tc.tile_pool(name="const", bufs=1) as const,
            tc.tile_pool(name="psum_proj", bufs=2, space="PSUM") as psum_proj,
            tc.tile_pool(name="psum_band", bufs=2, space="PSUM") as psum_band,
            tc.tile_pool(name="psum_sp", bufs=2, space="PSUM") as psum_sp,
            tc.tile_pool(name="band_e", bufs=2) as e_pool,
            tc.tile_pool(name="comb", bufs=1) as comb,
        ):
            # ---- engine warmups (run while DMAs are in flight) ----
            wtile = const.tile([P, 256], BF16, tag="warm_w")
            nc.vector.memset(wtile, 0.0)
            wact_in = const.tile([P, 1], F32, tag="warm_a")
            nc.vector.memset(wact_in, 0.0)
            for _ in range(N_WARM):
                wps = psum_band.tile([P, 2 * win], F32, tag="band")
                nc.tensor.matmul(
                    wps[:, :win], lhsT=wtile[:, :P], rhs=wtile[:, :win],
                    start=True, stop=True,
                )
            wact_out = const.tile([P, 1], F32, tag="warm_ao")
            nc.scalar.activation(out=wact_out, in_=wact_in, func=AF.Exp)

            # ---- input DMAs, ordered by need time ----
            # sync queue: xt0, cst16, cst (+ the y output later)
            # scalar queue: w2, xt1, xt2, xt3
            xts = []
            for i in range(NPC):
                t = const.tile([P, DCH * PROJ_CHUNK], F8, tag=f"xt{i}")
                xts.append(t)
            nc.sync.dma_start(out=xts[0], in_=xt_d[0])
            w2_s = const.tile([P, 2 * DCH * MD], F8, tag="w2")
            nc.scalar.dma_start(out=w2_s, in_=w2_d[:, :])
            c16_s = const.tile([P, (n_pat + 1) * 2 * win], BF16, tag="cst16")
            nc.sync.dma_start(out=c16_s, in_=c16_d[:, :])
            cs_s = const.tile([P, CW], F32, tag="cst")
            nc.sync.dma_start(out=cs_s, in_=cs_d[:, :])
            for i in range(1, NPC):
                nc.scalar.dma_start(out=xts[i], in_=xt_d[i])

            # q is cols [0, N), k is cols [N, 2N)
            qkT = const.tile([P, 2 * N], BF16, tag="qkT")
            # per-pair sums, 4 cols per pair: e(2g) | e(2g+1) | ec(2g) | ec(2g+1)
            sums4 = const.tile([P, 4 * NPAIR], F32, tag="sums4")
            outv = const.tile([P, NT], F32, tag="outv")

            # ---- projections: one DoubleRow matmul per (chunk, q/k),
            # both halves in one PSUM tile, halves evicted on DVE + ACT ----
            def emit_proj(n4):
                ps = psum_proj.tile([P, 2 * PROJ_CHUNK], F32, tag="proj")
                rhs = xts[n4][:, :].rearrange("p (c f) -> p c f", c=DCH)
                for pj in range(2):  # 0=q, 1=k
                    lhsT = w2_s[:, pj * DCH * MD : (pj + 1) * DCH * MD].rearrange(
                        "p (c m) -> p c m", c=DCH
                    )
                    nc.tensor.matmul(
                        ps[:, pj * PROJ_CHUNK : (pj + 1) * PROJ_CHUNK],
                        lhsT=lhsT,
                        rhs=rhs,
                        start=True,
                        stop=True,
                        perf_mode=mybir.MatmulPerfMode.DoubleRow,
                    )
                lo = n4 * PROJ_CHUNK
                if not has_bias:
                    # k first on ACT (bands need kT soonest), q on DVE
                    nc.scalar.activation(
                        out=qkT[:, N + lo : N + lo + PROJ_CHUNK],
                        in_=ps[:, PROJ_CHUNK:],
                        func=AF.Identity,
                    )
                    nc.vector.tensor_copy(
                        qkT[:, lo : lo + PROJ_CHUNK], ps[:, :PROJ_CHUNK]
                    )
                else:
                    for pj in range(2):
                        b_s = cs_s[:, O_BQ + pj : O_BQ + pj + 1]
                        nc.scalar.activation(
                            out=qkT[:, pj * N + lo : pj * N + lo + PROJ_CHUNK],
                            in_=ps[:, pj * PROJ_CHUNK : (pj + 1) * PROJ_CHUNK],
                            func=AF.Identity,
                            bias=b_s,
                            scale=1.0,
                        )

            # ---- band pair: tiles 2g, 2g+1 share one [P, 2*win] pass;
            # each tile is two 64-row halves on the partition dim ----
            pair_ps = {}

            def emit_pair_mm(g):
                ps = psum_band.tile([P, 2 * win], F32, tag="band")
                for tb in range(2):  # tile within pair
                    t = 2 * g + tb
                    for hb in range(2):  # 64-row half on partitions
                        ws = ws2[2 * t + hb]
                        nc.tensor.matmul(
                            ps[hb * HR : (hb + 1) * HR, tb * win : (tb + 1) * win],
                            lhsT=qkT[:, t * P + hb * HR : t * P + (hb + 1) * HR],
                            rhs=qkT[:, N + ws : N + ws + win],
                            start=True,
                            stop=True,
                        )
                pair_ps[g] = ps

            # reduce engine per pair (GpSimd cannot free-axis reduce)
            RED_GPS = set()

            def emit_pair_post(g):
                ps = pair_ps.pop(g)
                oi = key_idx[g]
                sp = psum_sp.tile([P, 2 * win], F32, tag="sp")
                nc.vector.tensor_mul(
                    sp, ps, c16_s[:, oi * 2 * win : (oi + 1) * 2 * win]
                )
                # packed e | ej tile: exp writes [:, :2win], gps ej [:, 2win:]
                eej = e_pool.tile([P, 4 * win], BF16, tag="eej")
                nc.scalar.activation(out=eej[:, : 2 * win], in_=sp, func=AF.Exp)
                nc.gpsimd.tensor_mul(
                    eej[:, 2 * win :],
                    eej[:, : 2 * win],
                    c16_s[:, O_J0 : O_J0 + 2 * win],
                )
                red_eng = nc.gpsimd if g in RED_GPS else nc.vector
                red_eng.tensor_reduce(
                    out=sums4[:, 4 * g : 4 * g + 4],
                    in_=eej[:].rearrange("p (f w) -> p f w", w=win),
                    axis=mybir.AxisListType.X,
                    op=AL.add,
                )

            # ---- combine: out = (A + sum_ec + B*sum_e)/(N-win+sum_e) ----
            # sums4 strided views: per pair 4 cols [e0, e1, ec0, ec1]
            def sview(sl, off):
                g0, g1 = sl.start // 2, sl.stop // 2
                return sums4[:, 4 * g0 : 4 * g1].rearrange(
                    "p (g four) -> p g four", four=4
                )[:, :, off : off + 2]

            def comb_pre(sl):
                w = sl.stop - sl.start
                se = sview(sl, 0)
                t0 = comb.tile([P, w], F32, tag=f"t0{sl.start}")
                nc.gpsimd.tensor_scalar_add(
                    t0[:].rearrange("p (g two) -> p g two", two=2),
                    se,
                    float(N - win),
                )
                rec = comb.tile([P, w], F32, tag=f"rec{sl.start}")
                nc.vector.reciprocal(rec, t0)
                tmp = comb.tile([P, w], F32, tag=f"tmp{sl.start}")
                nc.gpsimd.tensor_mul(
                    tmp[:].rearrange("p (g two) -> p g two", two=2),
                    cs_s[:, O_B + sl.start : O_B + sl.stop].rearrange(
                        "p (g two) -> p g two", two=2
                    ),
                    se,
                )
                return rec, tmp

            def comb_post(sl, rec, tmp):
                w = sl.stop - sl.start
                sec = sview(sl, 2)
                num = comb.tile([P, w], F32, tag=f"num{sl.start}")
                nc.gpsimd.tensor_add(
                    num[:].rearrange("p (g two) -> p g two", two=2),
                    cs_s[:, O_A + sl.start : O_A + sl.stop].rearrange(
                        "p (g two) -> p g two", two=2
                    ),
                    sec,
                )
                num2 = comb.tile([P, w], F32, tag=f"num2{sl.start}")
                nc.gpsimd.tensor_add(num2, num, tmp)
                nc.gpsimd.tensor_mul(outv[:, sl], num2, rec)
                nc.sync.dma_start(out=y_d[:, sl], in_=outv[:, sl])

            # ---- schedule ----
            def pair_chunk(g):
                hi = max(ws2[h] + win for h in range(4 * g, 4 * g + 4))
                return max((2 * g + 1) // (PROJ_CHUNK // P), (hi - 1) // PROJ_CHUNK)

            pairs_of = {c: [] for c in range(NPC)}
            for g in range(NPAIR):
                pairs_of[pair_chunk(g)].append(g)

            slA, slB = slice(0, 8), slice(8, NT)

            emit_proj(0)
            emit_proj(1)
            for g in pairs_of[0]:
                emit_pair_mm(g)
            emit_proj(2)
            for g in pairs_of[0]:
                emit_pair_post(g)
            for g in pairs_of[1]:
                emit_pair_mm(g)
            emit_proj(3)
            for g in pairs_of[1]:
                emit_pair_post(g)
            for g in pairs_of[2]:
                emit_pair_mm(g)
            for g in pairs_of[2]:
                emit_pair_post(g)
            for g in pairs_of[3]:
                emit_pair_mm(g)
            done = 0
            recA = tmpA = None
            npost = len(pairs_of[3])
            for g in pairs_of[3]:
                emit_pair_post(g)
                done += 1
                if done == npost - 1:
                    recA, tmpA = comb_pre(slA)
            comb_post(slA, recA, tmpA)
            recB, tmpB = comb_pre(slB)
            comb_post(slB, recB, tmpB)

    nc.finalize()
    return nc


def kernel(x, Wq, bq, Wk, bk, prior_mean, prior_std):
    global last_run
    x = np.asarray(x, dtype=np.float32)
    Wq = np.asarray(Wq, dtype=np.float32)
    Wk = np.asarray(Wk, dtype=np.float32)
    bq = np.asarray(bq, dtype=np.float32)
    bk = np.asarray(bk, dtype=np.float32)
    has_bias = bool(np.any(bq) or np.any(bk))

    prior, dlo, dhi = _plan_band(
        float(np.asarray(prior_mean)[0]), float(np.asarray(prior_std)[0])
    )
    win, ws2, key_vals, key_idx = _window_geometry(dlo, dhi)
    n_pat = len(key_vals)

    key = (win, tuple(ws2), tuple(key_idx), has_bias)
    if key not in _cache:
        _cache[key] = _build(win, ws2, key_idx, n_pat, has_bias)
    nc = _cache[key]

    bf = ml_dtypes.bfloat16
    f8 = ml_dtypes.float8_e4m3fn
    scale = np.float32(MD**-0.5)

    # prior*scale pair patterns: [P, 2*win] per distinct 4-offset key.
    # value[p, tb*win + c] = prior[c + rel_ws[tb, hb] - 128*tb - p] * scale
    # where hb selects by partition half (p >= 64).
    p_idx = np.arange(P)[:, None]
    c_idx = np.arange(win)[None, :]
    pmat = np.zeros((P, n_pat * 2 * win), np.float32)
    for ki, rel in enumerate(key_vals):
        for tb in range(2):
            relcol = np.where(np.arange(P) < HR, rel[2 * tb], rel[2 * tb + 1])[:, None]
            dm = c_idx + relcol - 128 * tb - p_idx
            pmat[:, ki * 2 * win + tb * win : ki * 2 * win + (tb + 1) * win] = np.where(
                (dm >= dlo) & (dm <= dhi), prior[dm + N - 1] * scale, np.float32(0.0)
            ).astype(np.float32)

    sumj_all = float(N * (N - 1) // 2)
    half_sel = np.arange(P) >= HR
    ii = (np.arange(P)[:, None] + P * np.arange(NT)[None, :]).astype(np.float32)
    wsv = np.zeros((P, NT), np.float32)
    for t in range(NT):
        wsv[:, t] = np.where(half_sel, float(ws2[2 * t + 1]), float(ws2[2 * t]))
    c1 = sumj_all - (win * wsv + win * (win - 1) // 2)
    A = c1 - ii * float(N - win)
    Bv = wsv - ii

    cst = np.ascontiguousarray(
        np.concatenate([bq.reshape(P, 1), bk.reshape(P, 1), A, Bv], axis=1).astype(
            np.float32
        )
    )
    j0pair = np.broadcast_to(
        np.tile(np.arange(win, dtype=np.float32), 2)[None, :], (P, 2 * win)
    )
    cst16 = np.ascontiguousarray(np.concatenate([pmat, j0pair], axis=1).astype(bf))

    # weights: wq chunks then wk chunks, [P, 4*MD] fp8
    wq_h = Wq.reshape(DCH, P, MD).transpose(1, 0, 2).reshape(P, DCH * MD)
    wk_h = Wk.reshape(DCH, P, MD).transpose(1, 0, 2).reshape(P, DCH * MD)
    w2_h = np.clip(np.concatenate([wq_h, wk_h], axis=1), -240, 240)
    w2_h = np.ascontiguousarray(w2_h).astype(f8)

    in_maps = []
    for core in range(NCORES):
        xb = x[core]  # [N, D]
        # xt[n4, p, c*512 + j] = x[n4*512 + j, c*128 + p]
        xt_h = np.ascontiguousarray(
            np.clip(xb.T, -240, 240)
            .reshape(DCH, P, NPC, PROJ_CHUNK)
            .transpose(2, 1, 0, 3)
            .reshape(NPC, P, DCH * PROJ_CHUNK)
        ).astype(f8)
        in_maps.append({"xt": xt_h, "w2": w2_h, "cst": cst, "cst16": cst16})

    res = run_bass_kernel_spmd(nc, in_maps, list(range(NCORES)))
    last_run = (nc, in_maps)
    # y[p, t] = out[128t + p]  ->  out = y.T.flatten()
    out = np.stack(
        [res.results[c]["y"].T.reshape(-1) for c in range(NCORES)], axis=0
    )
    return out.astype(np.float32)


# revision 8
# speedup vs baseline: 1.1278x; 1.0231x over previous
"""Trainium2 Bass kernel for nn_DistanceLayer (gaussian-prior distance attention).

Math: out[b,i] = sum_j softmax_j(q_i.k_j * MD^-0.5 * prior(j-i))[j] * (j-i)

The gaussian prior (std=1) underflows so fast in f32 that outside a small
band of offsets the f32 score is exactly 0, so exp(score) is exactly 1.0.
Each softmax row is a narrow band of interesting values plus a uniform far
field with closed-form sums.  We compute a narrow window of scores around
the diagonal on the PE and fold the far field in with exact constants:

    T0_i = (N - win) + sum_win e           (denominator)
    out_i = (A_i + sum_win e*j0 + B_i * sum_win e) / T0_i

with A_i = sum_all j - sum_win_i j - i*(N-win)  (exact ints in f32),
B_i = ws_i - i, j0 the window-local column index.

Structure:
- rows processed as 64-row halves packed two-per-partition-dim (windows
  stay narrow: win = 64 + band + pad); the h0/h64 half matmuls run
  concurrently on the PE via column groups.
- fp8 (e4m3) x and weights; each projection chunk is one DoubleRow matmul
  per q/k contracting all 256 input dims at 2 elem/cell; q and k land in
  one [P, 1024] PSUM tile, k evicted on ACT, q on DVE.
- postproc per QUAD of tiles [P, 4*win] to amortize fixed op costs:
  DVE multiplies scores by the premultiplied prior pattern into PSUM,
  ACT exp's the quad into a packed bf16 e|ej tile, GpSimd multiplies e by
  j0 into the ej half, and one DVE reduce over [P, (8, win)] yields all
  eight sums per quad.
- combine runs on GpSimd except the reciprocal; y is written in 2 DMAs.
- inputs split over the sync + scalar DMA queues ordered by need time.

Sharding: pure data-parallel over batch B=8 across the 8 cores.
"""

import sys

sys.path.insert(0, "/opt/trn_rl_repo")

import ml_dtypes
import numpy as np

import concourse.bacc as bacc
import concourse.tile as tile
from concourse import mybir
from concourse.bass_utils import run_bass_kernel_spmd

B, N, D, MD = 8, 2048, 256, 128
NCORES = 8
P = 128
HR = P // 2  # 64-row half-tiles
NT = N // P  # 16 row tiles
TPQ = 4  # tiles per postprocessing quad
NQUAD = NT // TPQ  # 4
DCH = D // P  # 2 contraction chunks (fused by DoubleRow)
PROJ_CHUNK = 512
NPC = N // PROJ_CHUNK  # 4 projection column chunks
PI = 3.1415926  # matches reference
F32 = mybir.dt.float32
BF16 = mybir.dt.bfloat16
F8 = mybir.dt.float8e4
AL = mybir.AluOpType
AF = mybir.ActivationFunctionType
N_WARM = 16  # PE clock-ramp junk matmuls

_cache = {}
# exposed for test harness profiling: (nc, in_maps)
last_run = None


def _plan_band(prior_mean, prior_std):
    """f32 prior over every offset, exactly as the reference computes it,
    and the band of offsets whose scores can round exp() away from 1.0."""
    d = np.arange(-(N - 1), N, dtype=np.float32)
    ps = np.float32(prior_std)
    pm = np.float32(prior_mean)
    prior = (
        np.float32(1.0)
        / ps
        / np.sqrt(np.float32(2.0) * np.float32(PI))
        * np.exp(np.float32(-0.5) * (d - pm) ** 2 / ps**2)
    ).astype(np.float32)
    sig = np.abs(prior) * 1024.0 >= 2.0**-27
    if not sig.any():
        dlo, dhi = 0, 0
    else:
        dlo = int(d[sig].min())
        dhi = int(d[sig].max())
    return prior, dlo, dhi


def _window_geometry(dlo, dhi):
    """Per-64-row-half window starts ws2[32] plus deduplicated per-quad
    prior patterns.  Pattern key for quad q (tiles 4q..4q+3) is the tuple
    of its eight half-window offsets relative to the quad's base row."""
    span = dhi - dlo
    win = HR + span + 1
    win = max(80, ((win + 15) // 16) * 16)
    assert win <= 128, f"prior band too wide for banded kernel: {dlo}..{dhi}"
    extra = win - (HR + span)
    ws2 = []
    for h in range(2 * NT):
        ws = min(max(h * HR + dlo - extra // 2, 0), N - win)
        lo_need = max(0, h * HR + dlo)
        hi_need = min(N - 1, h * HR + HR - 1 + dhi)
        assert ws <= lo_need and hi_need < ws + win, (h, ws, lo_need, hi_need)
        ws2.append(ws)
    quad_keys = []
    for q in range(NQUAD):
        base = TPQ * P * q
        quad_keys.append(tuple(ws2[2 * TPQ * q + i] - base for i in range(2 * TPQ)))
    key_vals = sorted(set(quad_keys))
    key_idx = [key_vals.index(k) for k in quad_keys]
    return win, ws2, key_vals, key_idx


def _build(win, ws2, key_idx, n_pat, has_bias):
    nc = bacc.Bacc()

    W4 = TPQ * win  # postproc pass width
    CW = 2 + 2 * NT
    O_BQ, O_BK = 0, 1
    O_A = 2
    O_B = O_A + NT
    O_J0 = n_pat * W4  # j0quad offset inside cst16

    w2_d = nc.dram_tensor("w2", [P, 2 * DCH * MD], F8, kind="ExternalInput")
    xt_d = nc.dram_tensor("xt", [NPC, P, DCH * PROJ_CHUNK], F8, kind="ExternalInput")
    cs_d = nc.dram_tensor("cst", [P, CW], F32, kind="ExternalInput")
    c16_d = nc.dram_tensor("cst16", [P, (n_pat + 1) * W4], BF16, kind="ExternalInput")
    y_d = nc.dram_tensor("y", [P, NT], F32, kind="ExternalOutput")

    with tile.TileContext(nc) as tc:
        with (
            tc.tile_pool(name="const", bufs=1) as const,
            tc.tile_pool(name="psum_proj", bufs=2, space="PSUM") as psum_proj,
            tc.tile_pool(name="psum_band", bufs=2, space="PSUM") as psum_band,
            tc.tile_pool(name="psum_sp", bufs=2, space="PSUM") as psum_sp,
            tc.tile_pool(name="band_e", bufs=2) as e_pool,
            tc.tile_pool(name="comb", bufs=1) as comb,
        ):
            # ---- engine warmups (run while DMAs are in flight) ----
            wtile = const.tile([P, 256], BF16, tag="warm_w")
            nc.vector.memset(wtile, 0.0)
            wact_in = const.tile([P, 1], F32, tag="warm_a")
            nc.vector.memset(wact_in, 0.0)
            for _ in range(N_WARM):
                wps = psum_band.tile([P, W4], F32, tag="band")
                nc.tensor.matmul(
                    wps[:, :256], lhsT=wtile[:, :P], rhs=wtile, start=True, stop=True
                )
            wact_out = const.tile([P, 1], F32, tag="warm_ao")
            nc.scalar.activation(out=wact_out, in_=wact_in, func=AF.Exp)

            # ---- input DMAs, ordered by need time ----
            # sync queue: xt0, xt2, cst16, cst (+ the y output later)
            # scalar queue: w2, xt1, xt3
            xts = []
            for i in range(NPC):
                t = const.tile([P, DCH * PROJ_CHUNK], F8, tag=f"xt{i}")
                xts.append(t)
            nc.sync.dma_start(out=xts[0], in_=xt_d[0])
            w2_s = const.tile([P, 2 * DCH * MD], F8, tag="w2")
            nc.scalar.dma_start(out=w2_s, in_=w2_d[:, :])
            nc.sync.dma_start(out=xts[2], in_=xt_d[2])
            nc.scalar.dma_start(out=xts[1], in_=xt_d[1])
            c16_s = const.tile([P, (n_pat + 1) * W4], BF16, tag="cst16")
            nc.sync.dma_start(out=c16_s, in_=c16_d[:, :])
            nc.scalar.dma_start(out=xts[3], in_=xt_d[3])
            cs_s = const.tile([P, CW], F32, tag="cst")
            nc.sync.dma_start(out=cs_s, in_=cs_d[:, :])

            # q is cols [0, N), k is cols [N, 2N)
            qkT = const.tile([P, 2 * N], BF16, tag="qkT")
            # per-quad sums, 8 cols per quad: e(4q..4q+3) | ec(4q..4q+3)
            sums8 = const.tile([P, 2 * TPQ * NQUAD], F32, tag="sums8")
            outv = const.tile([P, NT], F32, tag="outv")

            # ---- projections ----
            def emit_proj(n4):
                ps = psum_proj.tile([P, 2 * PROJ_CHUNK], F32, tag="proj")
                rhs = xts[n4][:, :].rearrange("p (c f) -> p c f", c=DCH)
                for pj in range(2):  # 0=q, 1=k
                    lhsT = w2_s[:, pj * DCH * MD : (pj + 1) * DCH * MD].rearrange(
                        "p (c m) -> p c m", c=DCH
                    )
                    nc.tensor.matmul(
                        ps[:, pj * PROJ_CHUNK : (pj + 1) * PROJ_CHUNK],
                        lhsT=lhsT,
                        rhs=rhs,
                        start=True,
                        stop=True,
                        perf_mode=mybir.MatmulPerfMode.DoubleRow,
                    )
                return ps

            def emit_evict(n4, ps, pj, eng):
                lo = n4 * PROJ_CHUNK
                src = ps[:, pj * PROJ_CHUNK : (pj + 1) * PROJ_CHUNK]
                dst = qkT[:, pj * N + lo : pj * N + lo + PROJ_CHUNK]
                if has_bias:
                    nc.scalar.activation(
                        out=dst, in_=src, func=AF.Identity,
                        bias=cs_s[:, O_BQ + pj : O_BQ + pj + 1], scale=1.0,
                    )
                elif eng == "act":
                    nc.scalar.activation(out=dst, in_=src, func=AF.Identity)
                else:
                    nc.vector.tensor_copy(dst, src)

            # ---- band quad: tiles 4q..4q+3 share one [P, 4*win] pass ----
            quad_ps = {}

            def emit_quad_mm(q):
                ps = psum_band.tile([P, W4], F32, tag="band")
                for tb in range(TPQ):
                    t = TPQ * q + tb
                    for hb in range(2):  # 64-row half on partitions
                        ws = ws2[2 * t + hb]
                        nc.tensor.matmul(
                            ps[hb * HR : (hb + 1) * HR, tb * win : (tb + 1) * win],
                            lhsT=qkT[:, t * P + hb * HR : t * P + (hb + 1) * HR],
                            rhs=qkT[:, N + ws : N + ws + win],
                            start=True,
                            stop=True,
                        )
                quad_ps[q] = ps

            def emit_quad_post(q):
                ps = quad_ps.pop(q)
                oi = key_idx[q]
                sp = psum_sp.tile([P, W4], F32, tag="sp")
                nc.vector.tensor_mul(sp, ps, c16_s[:, oi * W4 : (oi + 1) * W4])
                # packed e | ej tile: exp writes [:, :W4], gps ej [:, W4:]
                eej = e_pool.tile([P, 2 * W4], BF16, tag="eej")
                nc.scalar.activation(out=eej[:, :W4], in_=sp, func=AF.Exp)
                nc.gpsimd.tensor_mul(
                    eej[:, W4:], eej[:, :W4], c16_s[:, O_J0 : O_J0 + W4]
                )
                nc.vector.tensor_reduce(
                    out=sums8[:, 2 * TPQ * q : 2 * TPQ * (q + 1)],
                    in_=eej[:].rearrange("p (f w) -> p f w", w=win),
                    axis=mybir.AxisListType.X,
                    op=AL.add,
                )

            # ---- combine: out = (A + sum_ec + B*sum_e)/(N-win+sum_e) ----
            # sums8 views: per quad 8 cols [e x4, ec x4]
            def sview(sl, off):
                q0, q1 = sl.start // TPQ, sl.stop // TPQ
                return sums8[:, 2 * TPQ * q0 : 2 * TPQ * q1].rearrange(
                    "p (q eight) -> p q eight", eight=2 * TPQ
                )[:, :, off : off + TPQ]

            def cview(o, sl):
                return cs_s[:, o + sl.start : o + sl.stop].rearrange(
                    "p (q four) -> p q four", four=TPQ
                )

            def comb_pre(sl):
                w = sl.stop - sl.start
                se = sview(sl, 0)
                t0 = comb.tile([P, w], F32, tag=f"t0{sl.start}")
                nc.gpsimd.tensor_scalar_add(
                    t0[:].rearrange("p (q four) -> p q four", four=TPQ),
                    se,
                    float(N - win),
                )
                rec = comb.tile([P, w], F32, tag=f"rec{sl.start}")
                nc.vector.reciprocal(rec, t0)
                tmp = comb.tile([P, w], F32, tag=f"tmp{sl.start}")
                nc.gpsimd.tensor_mul(
                    tmp[:].rearrange("p (q four) -> p q four", four=TPQ),
                    cview(O_B, sl),
                    se,
                )
                return rec, tmp

            def comb_post(sl, rec, tmp):
                w = sl.stop - sl.start
                sec = sview(sl, TPQ)
                num = comb.tile([P, w], F32, tag=f"num{sl.start}")
                nc.gpsimd.tensor_add(
                    num[:].rearrange("p (q four) -> p q four", four=TPQ),
                    cview(O_A, sl),
                    sec,
                )
                num2 = comb.tile([P, w], F32, tag=f"num2{sl.start}")
                nc.gpsimd.tensor_add(num2, num, tmp)
                nc.gpsimd.tensor_mul(outv[:, sl], num2, rec)
                nc.sync.dma_start(out=y_d[:, sl], in_=outv[:, sl])

            # ---- schedule ----
            slA, slB = slice(0, 8), slice(8, NT)

            ps0 = emit_proj(0)
            emit_evict(0, ps0, 1, "act")
            emit_evict(0, ps0, 0, "dve")
            ps1 = emit_proj(1)
            emit_evict(1, ps1, 1, "act")
            emit_evict(1, ps1, 0, "dve")
            ps2 = emit_proj(2)
            emit_evict(2, ps2, 1, "act")
            emit_quad_mm(0)
            ps3 = emit_proj(3)
            emit_evict(3, ps3, 1, "act")
            emit_evict(2, ps2, 0, "dve")
            emit_evict(3, ps3, 0, "dve")
            emit_quad_post(0)
            emit_quad_mm(1)
            emit_quad_post(1)
            emit_quad_mm(2)
            recA, tmpA = comb_pre(slA)
            comb_post(slA, recA, tmpA)
            emit_quad_post(2)
            emit_quad_mm(3)
            emit_quad_post(3)
            recB, tmpB = comb_pre(slB)
            comb_post(slB, recB, tmpB)

    nc.finalize()
    return nc


def kernel(x, Wq, bq, Wk, bk, prior_mean, prior_std):
    global last_run
    x = np.asarray(x, dtype=np.float32)
    Wq = np.asarray(Wq, dtype=np.float32)
    Wk = np.asarray(Wk, dtype=np.float32)
    bq = np.asarray(bq, dtype=np.float32)
    bk = np.asarray(bk, dtype=np.float32)
    has_bias = bool(np.any(bq) or np.any(bk))

    prior, dlo, dhi = _plan_band(
        float(np.asarray(prior_mean)[0]), float(np.asarray(prior_std)[0])
    )
    win, ws2, key_vals, key_idx = _window_geometry(dlo, dhi)
    n_pat = len(key_vals)

    key = (win, tuple(ws2), tuple(key_idx), has_bias)
    if key not in _cache:
        _cache[key] = _build(win, ws2, key_idx, n_pat, has_bias)
    nc = _cache[key]

    bf = ml_dtypes.bfloat16
    f8 = ml_dtypes.float8_e4m3fn
    scale = np.float32(MD**-0.5)
    W4 = TPQ * win

    # prior*scale quad patterns: [P, 4*win] per distinct 8-offset key.
    # value[p, tb*win + c] = prior[c + rel_ws[tb, hb] - 128*tb - p] * scale
    # where hb selects by partition half (p >= 64).
    p_idx = np.arange(P)[:, None]
    c_idx = np.arange(win)[None, :]
    pmat = np.zeros((P, n_pat * W4), np.float32)
    for ki, rel in enumerate(key_vals):
        for tb in range(TPQ):
            relcol = np.where(np.arange(P) < HR, rel[2 * tb], rel[2 * tb + 1])[:, None]
            dm = c_idx + relcol - 128 * tb - p_idx
            pmat[:, ki * W4 + tb * win : ki * W4 + (tb + 1) * win] = np.where(
                (dm >= dlo) & (dm <= dhi), prior[dm + N - 1] * scale, np.float32(0.0)
            ).astype(np.float32)

    sumj_all = float(N * (N - 1) // 2)
    half_sel = np.arange(P) >= HR
    ii = (np.arange(P)[:, None] + P * np.arange(NT)[None, :]).astype(np.float32)
    wsv = np.zeros((P, NT), np.float32)
    for t in range(NT):
        wsv[:, t] = np.where(half_sel, float(ws2[2 * t + 1]), float(ws2[2 * t]))
    c1 = sumj_all - (win * wsv + win * (win - 1) // 2)
    A = c1 - ii * float(N - win)
    Bv = wsv - ii

    cst = np.ascontiguousarray(
        np.concatenate([bq.reshape(P, 1), bk.reshape(P, 1), A, Bv], axis=1).astype(
            np.float32
        )
    )
    j0quad = np.broadcast_to(
        np.tile(np.arange(win, dtype=np.float32), TPQ)[None, :], (P, W4)
    )
    cst16 = np.ascontiguousarray(np.concatenate([pmat, j0quad], axis=1).astype(bf))

    # weights: wq chunks then wk chunks, [P, 4*MD] fp8
    wq_h = Wq.reshape(DCH, P, MD).transpose(1, 0, 2).reshape(P, DCH * MD)
    wk_h = Wk.reshape(DCH, P, MD).transpose(1, 0, 2).reshape(P, DCH * MD)
    w2_h = np.clip(np.concatenate([wq_h, wk_h], axis=1), -240, 240)
    w2_h = np.ascontiguousarray(w2_h).astype(f8)

    in_maps = []
    for core in range(NCORES):
        xb = x[core]  # [N, D]
        # xt[n4, p, c*512 + j] = x[n4*512 + j, c*128 + p]
        xt_h = np.ascontiguousarray(
            np.clip(xb.T, -240, 240)
            .reshape(DCH, P, NPC, PROJ_CHUNK)
            .transpose(2, 1, 0, 3)
            .reshape(NPC, P, DCH * PROJ_CHUNK)
        ).astype(f8)
        in_maps.append({"xt": xt_h, "w2": w2_h, "cst": cst, "cst16": cst16})

    res = run_bass_kernel_spmd(nc, in_maps, list(range(NCORES)))
    last_run = (nc, in_maps)
    # y[p, t] = out[128t + p]  ->  out = y.T.flatten()
    out = np.stack(
        [res.results[c]["y"].T.reshape(-1) for c in range(NCORES)], axis=0
    )
    return out.astype(np.float32)


# revision 10
# speedup vs baseline: 1.1428x; 1.0133x over previous
"""Trainium2 Bass kernel for nn_DistanceLayer (gaussian-prior distance attention).

Math: out[b,i] = sum_j softmax_j(q_i.k_j * MD^-0.5 * prior(j-i))[j] * (j-i)

The gaussian prior (std=1) underflows so fast in f32 that outside a small
band of offsets the f32 score is exactly 0, so exp(score) is exactly 1.0.
Each softmax row is a narrow band of interesting values plus a uniform far
field with closed-form sums.  We compute a narrow window of scores around
the diagonal on the PE and fold the far field in with exact constants:

    T0_i = (N - win) + sum_win e           (denominator)
    out_i = (A_i + sum_win e*j0 + B_i * sum_win e) / T0_i

with A_i = sum_all j - sum_win_i j - i*(N-win)  (exact ints in f32),
B_i = ws_i - i, j0 the window-local column index.

Structure:
- rows processed as 64-row halves packed two-per-partition-dim (windows
  stay narrow: win = 64 + band + pad); the h0/h64 half matmuls run
  concurrently on the PE via column groups.
- fp8 (e4m3) x and weights; each projection chunk is one DoubleRow matmul
  per q/k contracting all 256 input dims at 2 elem/cell; q and k land in
  one [P, 1024] PSUM tile, k evicted on ACT, q on DVE.
- postproc per QUAD of tiles [P, 4*win] to amortize fixed op costs:
  DVE multiplies scores by the premultiplied prior pattern into PSUM,
  ACT exp's the quad into a packed bf16 e|ej tile, GpSimd multiplies e by
  j0 into the ej half, and one DVE reduce over [P, (8, win)] yields all
  eight sums per quad.
- combine runs on GpSimd except the reciprocal; y is written in 2 DMAs.
- inputs split over the sync + scalar DMA queues ordered by need time.

Sharding: pure data-parallel over batch B=8 across the 8 cores.
"""

import sys

sys.path.insert(0, "/opt/trn_rl_repo")

import ml_dtypes
import numpy as np

import concourse.bacc as bacc
import concourse.tile as tile
from concourse import mybir
from concourse.bass_utils import run_bass_kernel_spmd

B, N, D, MD = 8, 2048, 256, 128
NCORES = 8
P = 128
HR = P // 2  # 64-row half-tiles
NT = N // P  # 16 row tiles
TPQ = 4  # tiles per postprocessing quad
NQUAD = NT // TPQ  # 4
DCH = D // P  # 2 contraction chunks (fused by DoubleRow)
PROJ_CHUNK = 512
NPC = N // PROJ_CHUNK  # 4 projection column chunks
PI = 3.1415926  # matches reference
F32 = mybir.dt.float32
BF16 = mybir.dt.bfloat16
F8 = mybir.dt.float8e4
AL = mybir.AluOpType
AF = mybir.ActivationFunctionType
N_WARM = 26  # PE clock-ramp junk matmuls

_cache = {}
# exposed for test harness profiling: (nc, in_maps)
last_run = None


def _plan_band(prior_mean, prior_std):
    """f32 prior over every offset, exactly as the reference computes it,
    and the band of offsets whose scores can round exp() away from 1.0."""
    d = np.arange(-(N - 1), N, dtype=np.float32)
    ps = np.float32(prior_std)
    pm = np.float32(prior_mean)
    prior = (
        np.float32(1.0)
        / ps
        / np.sqrt(np.float32(2.0) * np.float32(PI))
        * np.exp(np.float32(-0.5) * (d - pm) ** 2 / ps**2)
    ).astype(np.float32)
    sig = np.abs(prior) * 1024.0 >= 2.0**-27
    if not sig.any():
        dlo, dhi = 0, 0
    else:
        dlo = int(d[sig].min())
        dhi = int(d[sig].max())
    return prior, dlo, dhi


def _window_geometry(dlo, dhi):
    """Per-64-row-half window starts ws2[32] plus deduplicated per-quad
    prior patterns.  Pattern key for quad q (tiles 4q..4q+3) is the tuple
    of its eight half-window offsets relative to the quad's base row."""
    span = dhi - dlo
    win = HR + span + 1
    win = max(80, ((win + 15) // 16) * 16)
    assert win <= 128, f"prior band too wide for banded kernel: {dlo}..{dhi}"
    extra = win - (HR + span)
    ws2 = []
    for h in range(2 * NT):
        ws = min(max(h * HR + dlo - extra // 2, 0), N - win)
        lo_need = max(0, h * HR + dlo)
        hi_need = min(N - 1, h * HR + HR - 1 + dhi)
        assert ws <= lo_need and hi_need < ws + win, (h, ws, lo_need, hi_need)
        ws2.append(ws)
    quad_keys = []
    for q in range(NQUAD):
        base = TPQ * P * q
        quad_keys.append(tuple(ws2[2 * TPQ * q + i] - base for i in range(2 * TPQ)))
    key_vals = sorted(set(quad_keys))
    key_idx = [key_vals.index(k) for k in quad_keys]
    return win, ws2, key_vals, key_idx


def _build(win, ws2, key_idx, n_pat, has_bias):
    nc = bacc.Bacc()

    W4 = TPQ * win  # postproc pass width
    CW = 2 + 2 * NT
    O_BQ, O_BK = 0, 1
    O_A = 2
    O_B = O_A + NT
    O_J0 = n_pat * W4  # j0quad offset inside cst16

    w2_d = nc.dram_tensor("w2", [P, 2 * DCH * MD], F8, kind="ExternalInput")
    xt_d = nc.dram_tensor("xt", [NPC, P, DCH * PROJ_CHUNK], F8, kind="ExternalInput")
    cs_d = nc.dram_tensor("cst", [P, CW], F32, kind="ExternalInput")
    c16_d = nc.dram_tensor("cst16", [P, (n_pat + 1) * W4], BF16, kind="ExternalInput")
    y_d = nc.dram_tensor("y", [P, NT], F32, kind="ExternalOutput")

    with tile.TileContext(nc) as tc:
        with (
            tc.tile_pool(name="const", bufs=1) as const,
            tc.tile_pool(name="psum_proj", bufs=2, space="PSUM") as psum_proj,
            tc.tile_pool(name="psum_band", bufs=2, space="PSUM") as psum_band,
            tc.tile_pool(name="psum_sp", bufs=2, space="PSUM") as psum_sp,
            tc.tile_pool(name="band_e", bufs=2) as e_pool,
            tc.tile_pool(name="comb", bufs=1) as comb,
        ):
            # ---- engine warmups (run while DMAs are in flight) ----
            wtile = const.tile([P, 64], BF16, tag="warm_w")
            nc.vector.memset(wtile, 0.0)
            wact_in = const.tile([P, 1], F32, tag="warm_a")
            nc.vector.memset(wact_in, 0.0)
            for _ in range(N_WARM):
                wps = psum_band.tile([P, W4], F32, tag="band")
                nc.tensor.matmul(
                    wps[:64, :64], lhsT=wtile[:, :64], rhs=wtile, start=True, stop=True
                )
            wact_out = const.tile([P, 1], F32, tag="warm_ao")
            nc.scalar.activation(out=wact_out, in_=wact_in, func=AF.Exp)

            # ---- input DMAs, ordered by need time ----
            # sync queue: xt0, xt2, cst16, cst (+ the y output later)
            # scalar queue: w2, xt1, xt3
            xts = []
            for i in range(NPC):
                t = const.tile([P, DCH * PROJ_CHUNK], F8, tag=f"xt{i}")
                xts.append(t)
            nc.sync.dma_start(out=xts[0], in_=xt_d[0])
            w2_s = const.tile([P, 2 * DCH * MD], F8, tag="w2")
            nc.scalar.dma_start(out=w2_s, in_=w2_d[:, :])
            nc.sync.dma_start(out=xts[2], in_=xt_d[2])
            nc.scalar.dma_start(out=xts[1], in_=xt_d[1])
            c16_s = const.tile([P, (n_pat + 1) * W4], BF16, tag="cst16")
            nc.sync.dma_start(out=c16_s, in_=c16_d[:, :])
            nc.scalar.dma_start(out=xts[3], in_=xt_d[3])
            cs_s = const.tile([P, CW], F32, tag="cst")
            nc.sync.dma_start(out=cs_s, in_=cs_d[:, :])

            # q is cols [0, N), k is cols [N, 2N)
            qkT = const.tile([P, 2 * N], BF16, tag="qkT")
            # per-quad sums, 8 cols per quad: e(4q..4q+3) | ec(4q..4q+3)
            sums8 = const.tile([P, 2 * TPQ * NQUAD], F32, tag="sums8")
            outv = const.tile([P, NT], F32, tag="outv")

            # ---- projections ----
            def emit_proj(n4):
                ps = psum_proj.tile([P, 2 * PROJ_CHUNK], F32, tag="proj")
                rhs = xts[n4][:, :].rearrange("p (c f) -> p c f", c=DCH)
                for pj in range(2):  # 0=q, 1=k
                    lhsT = w2_s[:, pj * DCH * MD : (pj + 1) * DCH * MD].rearrange(
                        "p (c m) -> p c m", c=DCH
                    )
                    nc.tensor.matmul(
                        ps[:, pj * PROJ_CHUNK : (pj + 1) * PROJ_CHUNK],
                        lhsT=lhsT,
                        rhs=rhs,
                        start=True,
                        stop=True,
                        perf_mode=mybir.MatmulPerfMode.DoubleRow,
                    )
                return ps

            def emit_evict(n4, ps, pj, eng):
                lo = n4 * PROJ_CHUNK
                src = ps[:, pj * PROJ_CHUNK : (pj + 1) * PROJ_CHUNK]
                dst = qkT[:, pj * N + lo : pj * N + lo + PROJ_CHUNK]
                if has_bias:
                    nc.scalar.activation(
                        out=dst, in_=src, func=AF.Identity,
                        bias=cs_s[:, O_BQ + pj : O_BQ + pj + 1], scale=1.0,
                    )
                elif eng == "act":
                    nc.scalar.activation(out=dst, in_=src, func=AF.Identity)
                else:
                    nc.vector.tensor_copy(dst, src)

            # ---- band quad: tiles 4q..4q+3 share one [P, 4*win] pass ----
            quad_ps = {}

            def emit_quad_mm(q):
                ps = psum_band.tile([P, W4], F32, tag="band")
                for tb in range(TPQ):
                    t = TPQ * q + tb
                    for hb in range(2):  # 64-row half on partitions
                        ws = ws2[2 * t + hb]
                        nc.tensor.matmul(
                            ps[hb * HR : (hb + 1) * HR, tb * win : (tb + 1) * win],
                            lhsT=qkT[:, t * P + hb * HR : t * P + (hb + 1) * HR],
                            rhs=qkT[:, N + ws : N + ws + win],
                            start=True,
                            stop=True,
                        )
                quad_ps[q] = ps

            def emit_quad_post(q):
                ps = quad_ps.pop(q)
                oi = key_idx[q]
                sp = psum_sp.tile([P, W4], F32, tag="sp")
                nc.vector.tensor_mul(sp, ps, c16_s[:, oi * W4 : (oi + 1) * W4])
                # packed e | ej tile: exp writes [:, :W4], gps ej [:, W4:]
                eej = e_pool.tile([P, 2 * W4], BF16, tag="eej")
                nc.scalar.activation(out=eej[:, :W4], in_=sp, func=AF.Exp)
                nc.vector.tensor_mul(
                    eej[:, W4:], eej[:, :W4], c16_s[:, O_J0 : O_J0 + W4]
                )
                nc.vector.tensor_reduce(
                    out=sums8[:, 2 * TPQ * q : 2 * TPQ * (q + 1)],
                    in_=eej[:].rearrange("p (f w) -> p f w", w=win),
                    axis=mybir.AxisListType.X,
                    op=AL.add,
                )

            # ---- combine: out = (A + sum_ec + B*sum_e)/(N-win+sum_e) ----
            # sums8 views: per quad 8 cols [e x4, ec x4]
            def sview(sl, off):
                q0, q1 = sl.start // TPQ, sl.stop // TPQ
                return sums8[:, 2 * TPQ * q0 : 2 * TPQ * q1].rearrange(
                    "p (q eight) -> p q eight", eight=2 * TPQ
                )[:, :, off : off + TPQ]

            def cview(o, sl):
                return cs_s[:, o + sl.start : o + sl.stop].rearrange(
                    "p (q four) -> p q four", four=TPQ
                )

            def comb_pre(sl):
                w = sl.stop - sl.start
                se = sview(sl, 0)
                t0 = comb.tile([P, w], F32, tag=f"t0{sl.start}")
                nc.gpsimd.tensor_scalar_add(
                    t0[:].rearrange("p (q four) -> p q four", four=TPQ),
                    se,
                    float(N - win),
                )
                rec = comb.tile([P, w], F32, tag=f"rec{sl.start}")
                nc.vector.reciprocal(rec, t0)
                tmp = comb.tile([P, w], F32, tag=f"tmp{sl.start}")
                nc.gpsimd.tensor_mul(
                    tmp[:].rearrange("p (q four) -> p q four", four=TPQ),
                    cview(O_B, sl),
                    se,
                )
                return rec, tmp

            def comb_post(sl, rec, tmp):
                w = sl.stop - sl.start
                sec = sview(sl, TPQ)
                num = comb.tile([P, w], F32, tag=f"num{sl.start}")
                nc.gpsimd.tensor_add(
                    num[:].rearrange("p (q four) -> p q four", four=TPQ),
                    cview(O_A, sl),
                    sec,
                )
                num2 = comb.tile([P, w], F32, tag=f"num2{sl.start}")
                nc.gpsimd.tensor_add(num2, num, tmp)
                nc.gpsimd.tensor_mul(outv[:, sl], num2, rec)
                nc.sync.dma_start(out=y_d[:, sl], in_=outv[:, sl])

            # ---- schedule ----
            slA, slB = slice(0, 8), slice(8, NT)

            ps0 = emit_proj(0)
            emit_evict(0, ps0, 1, "act")
            emit_evict(0, ps0, 0, "dve")
            ps1 = emit_proj(1)
            emit_evict(1, ps1, 1, "act")
            emit_evict(1, ps1, 0, "dve")
            ps2 = emit_proj(2)
            emit_evict(2, ps2, 1, "act")
            emit_quad_mm(0)
            ps3 = emit_proj(3)
            emit_evict(2, ps2, 0, "act")
            emit_evict(3, ps3, 1, "act")
            emit_quad_post(0)
            emit_quad_mm(1)
            emit_evict(3, ps3, 0, "act")
            emit_quad_post(1)
            emit_quad_mm(2)
            recA, tmpA = comb_pre(slA)
            comb_post(slA, recA, tmpA)
            emit_quad_post(2)
            emit_quad_mm(3)
            emit_quad_post(3)
            recB, tmpB = comb_pre(slB)
            comb_post(slB, recB, tmpB)

    nc.finalize()
    return nc


def kernel(x, Wq, bq, Wk, bk, prior_mean, prior_std):
    global last_run
    x = np.asarray(x, dtype=np.float32)
    Wq = np.asarray(Wq, dtype=np.float32)
    Wk = np.asarray(Wk, dtype=np.float32)
    bq = np.asarray(bq, dtype=np.float32)
    bk = np.asarray(bk, dtype=np.float32)
    has_bias = bool(np.any(bq) or np.any(bk))

    prior, dlo, dhi = _plan_band(
        float(np.asarray(prior_mean)[0]), float(np.asarray(prior_std)[0])
    )
    win, ws2, key_vals, key_idx = _window_geometry(dlo, dhi)
    n_pat = len(key_vals)

    key = (win, tuple(ws2), tuple(key_idx), has_bias)
    if key not in _cache:
        _cache[key] = _build(win, ws2, key_idx, n_pat, has_bias)
    nc = _cache[key]

    bf = ml_dtypes.bfloat16
    f8 = ml_dtypes.float8_e4m3fn
    scale = np.float32(MD**-0.5)
    W4 = TPQ * win

    # prior*scale quad patterns: [P, 4*win] per distinct 8-offset key.
    # value[p, tb*win + c] = prior[c + rel_ws[tb, hb] - 128*tb - p] * scale
    # where hb selects by partition half (p >= 64).
    p_idx = np.arange(P)[:, None]
    c_idx = np.arange(win)[None, :]
    pmat = np.zeros((P, n_pat * W4), np.float32)
    for ki, rel in enumerate(key_vals):
        for tb in range(TPQ):
            relcol = np.where(np.arange(P) < HR, rel[2 * tb], rel[2 * tb + 1])[:, None]
            dm = c_idx + relcol - 128 * tb - p_idx
            pmat[:, ki * W4 + tb * win : ki * W4 + (tb + 1) * win] = np.where(
                (dm >= dlo) & (dm <= dhi), prior[dm + N - 1] * scale, np.float32(0.0)
            ).astype(np.float32)

    sumj_all = float(N * (N - 1) // 2)
    half_sel = np.arange(P) >= HR
    ii = (np.arange(P)[:, None] + P * np.arange(NT)[None, :]).astype(np.float32)
    wsv = np.zeros((P, NT), np.float32)
    for t in range(NT):
        wsv[:, t] = np.where(half_sel, float(ws2[2 * t + 1]), float(ws2[2 * t]))
    c1 = sumj_all - (win * wsv + win * (win - 1) // 2)
    A = c1 - ii * float(N - win)
    Bv = wsv - ii

    cst = np.ascontiguousarray(
        np.concatenate([bq.reshape(P, 1), bk.reshape(P, 1), A, Bv], axis=1).astype(
            np.float32
        )
    )
    j0quad = np.broadcast_to(
        np.tile(np.arange(win, dtype=np.float32), TPQ)[None, :], (P, W4)
    )
    cst16 = np.ascontiguousarray(np.concatenate([pmat, j0quad], axis=1).astype(bf))

    # weights: wq chunks then wk chunks, [P, 4*MD] fp8
    wq_h = Wq.reshape(DCH, P, MD).transpose(1, 0, 2).reshape(P, DCH * MD)
    wk_h = Wk.reshape(DCH, P, MD).transpose(1, 0, 2).reshape(P, DCH * MD)
    w2_h = np.clip(np.concatenate([wq_h, wk_h], axis=1), -240, 240)
    w2_h = np.ascontiguousarray(w2_h).astype(f8)

    in_maps = []
    for core in range(NCORES):
        xb = x[core]  # [N, D]
        # xt[n4, p, c*512 + j] = x[n4*512 + j, c*128 + p]
        xt_h = np.ascontiguousarray(
            np.clip(xb.T, -240, 240)
            .reshape(DCH, P, NPC, PROJ_CHUNK)
            .transpose(2, 1, 0, 3)
            .reshape(NPC, P, DCH * PROJ_CHUNK)
        ).astype(f8)
        in_maps.append({"xt": xt_h, "w2": w2_h, "cst": cst, "cst16": cst16})

    res = run_bass_kernel_spmd(nc, in_maps, list(range(NCORES)))
    last_run = (nc, in_maps)
    # y[p, t] = out[128t + p]  ->  out = y.T.flatten()
    out = np.stack(
        [res.results[c]["y"].T.reshape(-1) for c in range(NCORES)], axis=0
    )
    return out.astype(np.float32)
